# revision 1
# baseline (speedup 1.0000x reference)
"""Trainium2 Bass kernel for nn_DeChunkLayer (ragged_sequence).

Math (per batch row, all in fp32):
    p[c]     = clip(boundary_prob[take_idx[c]], EPS, 1-EPS)
    decay    = 1 - p, decay[0] = 0
    weighted = p * hidden, weighted[0] = hidden[0]
    smoothed[c] = decay[c] * smoothed[c-1] + weighted[c]      (EMA scan over C)
    chunk_id = clip(cumsum(boundary_mask) - 1, 0, C-1)
    out[l]   = smoothed[chunk_id[l]]

Device mapping (1 batch row per NeuronCore, 8 cores):
  - p gather: 16x indirect DMA of [128, 1] scalars (the DGE here only
    supports one dynamic offset per partition) into a [128, 16] tile
    (c = j*128 + p), processed there, PE-transposed to a contiguous row
    and DRAM-bounced into [128, C] broadcast tiles.
  - EMA scan: DVE tensor_tensor_scan in a transposed [D-part, C-free]
    layout; hidden arrives pre-transposed ([D, C]) from the host shard.
  - smoothed is transposed back to [C, D] with 64 PE transposes and
    written to a DRAM scratch tensor.
  - chunk ids are computed directly in dma_gather's wrapped-16 layout
    CK16[q, j] = chunk_id[16j + q] (the host pre-wraps the mask): a
    16x16 inclusive-triangular matmul gives within-column prefixes, a
    1-partition scan of the column sums gives the across-column offsets
    (accumulated into the same PSUM bank via a ones-broadcast matmul).
  - output expansion: 8x dma_gather of 1024 2KB rows each; with the
    wrapped index layout, call k's slot s is exactly token 1024k + s,
    so each gathered [128, 8, 512] tile writes back as a plain
    (g p) d -> p g d view with 2KB-contiguous descriptors.
Tile does not emit RAW waits through DRAM scratch tensors, so every
DRAM-bounce reader carries an explicit sync dep on its writer.
"""

import numpy as np

import concourse.bass as bass
import concourse.bacc as bacc
import concourse.mybir as mybir
import concourse.tile as tile
from concourse.bass_utils import run_bass_kernel_spmd
from concourse.masks import make_identity, make_upper_triangular

B, L, C, D = 8, 8192, 2048, 512
EPS = 1e-4
P = 128
NDG = D // P          # 4 partition groups of the transposed layout
NCB = C // P          # 16 c-blocks of 128
NPJ = C // P          # 16 p-gather calls
QW = 16               # wrap width of the dma_gather index layout
FW = L // QW          # 512 free positions in the wrapped layout
NGC = 8               # output dma_gather calls
IPC = L // NGC        # 1024 indices per call

F32 = mybir.dt.float32
I16 = mybir.dt.int16
I32 = mybir.dt.int32
U8 = mybir.dt.uint8

_CACHED_NC = None


def build_nc() -> bacc.Bacc:
    nc = bacc.Bacc("TRN2", target_bir_lowering=False, debug=False)

    hidden_t = nc.dram_tensor("hidden_t", [D, C], F32, kind="ExternalInput")
    maskw_d = nc.dram_tensor("maskw", [QW, FW], U8, kind="ExternalInput")
    prob_d = nc.dram_tensor("prob", [L, 1], F32, kind="ExternalInput")
    tidx_d = nc.dram_tensor("tidx", [P, NPJ], I32, kind="ExternalInput")
    out_d = nc.dram_tensor("out", [L, D], F32, kind="ExternalOutput")

    with tile.TileContext(nc) as tc:
        with (
            tc.tile_pool(name="persist", bufs=1) as pp,
            tc.tile_pool(name="gather", bufs=3) as gp,
            tc.tile_pool(name="psum", bufs=4, space="PSUM") as psp,
            tc.tile_pool(name="psone", bufs=1, space="PSUM") as psone,
            tc.tile_pool(name="dram", bufs=1, space="DRAM") as dp,
        ):
            # ---------------- p gather (first: it gates the scan chain) ----
            tidx_sb = pp.tile([P, NPJ], I32)
            nc.scalar.dma_start(tidx_sb[:], tidx_d[:])
            p2 = pp.tile([P, NPJ], F32)   # (p, j) = p value for c = j*128 + p
            for j in range(NPJ):
                nc.gpsimd.indirect_dma_start(
                    out=p2[:, j:j + 1], out_offset=None, in_=prob_d[:],
                    in_offset=bass.IndirectOffsetOnAxis(
                        ap=tidx_sb[:, j:j + 1], axis=0))
            nc.vector.tensor_scalar(p2[:], p2[:], EPS, 1.0 - EPS,
                                    mybir.AluOpType.max, mybir.AluOpType.min)
            d2 = pp.tile([P, NPJ], F32)
            nc.vector.tensor_scalar(d2[:], p2[:], -1.0, 1.0,
                                    mybir.AluOpType.mult, mybir.AluOpType.add)
            nc.vector.memset(d2[0:1, 0:1], 0.0)   # decay[0] = 0
            nc.vector.memset(p2[0:1, 0:1], 1.0)   # weighted[0] = hidden[0]

            # ---------------- constants ----------------
            # Masks built on Pool, then bounced through a DVE copy so matmuls
            # consuming them wait on a single (DVE) semaphore.
            ident_p = pp.tile([P, P], F32)
            make_identity(nc, ident_p[:])
            uti_p = pp.tile([QW, QW], F32)
            make_upper_triangular(nc, uti_p[:], val=1.0, diag=True)  # k <= m
            ident = pp.tile([P, P], F32)
            nc.vector.tensor_copy(ident[:], ident_p[:])
            uti = pp.tile([QW, QW], F32)
            nc.vector.tensor_copy(uti[:], uti_p[:])
            ones16 = pp.tile([1, QW], F32)
            nc.vector.memset(ones16[:], 1.0)

            # p/decay rows: transpose to [16, 128] (contiguous c per
            # partition), write to DRAM, broadcast-read into [128, C].
            pT_ps = psone.tile([NPJ, P], F32, space="PSUM", tag="prow")
            nc.tensor.transpose(pT_ps[:], p2[:], ident[:])
            dT_ps = psone.tile([NPJ, P], F32, space="PSUM", tag="drow")
            nc.tensor.transpose(dT_ps[:], d2[:], ident[:])
            pT = pp.tile([NPJ, P], F32)
            nc.vector.tensor_copy(pT[:], pT_ps[:])
            dT = pp.tile([NPJ, P], F32)
            nc.vector.tensor_copy(dT[:], dT_ps[:])

            p_dram = dp.tile([1, C], F32)
            d_dram = dp.tile([1, C], F32)
            w_p = nc.scalar.dma_start(
                p_dram[:].rearrange("o (j q) -> (o j) q", j=NPJ), pT[:])
            w_d = nc.scalar.dma_start(
                d_dram[:].rearrange("o (j q) -> (o j) q", j=NPJ), dT[:])
            pb = pp.tile([P, C], F32)
            db = pp.tile([P, C], F32)
            r_p = nc.scalar.dma_start(pb[:], p_dram[:].to_broadcast([P, C]))
            r_d = nc.scalar.dma_start(db[:], d_dram[:].to_broadcast([P, C]))
            bass._add_dep_helper(r_p.ins, w_p.ins, sync=True, reason="p bounce raw")
            bass._add_dep_helper(r_d.ins, w_d.ins, sync=True, reason="d bounce raw")

            # ---------------- EMA scan in transposed layout ----------------
            sm_sb = pp.tile([P, NCB * D], F32)  # [c-in-block, (c-block, d)]
            sts = []
            for dg in range(NDG):
                ht = pp.tile([P, C], F32, tag=f"ht{dg}")
                nc.sync.dma_start(ht[:], hidden_t[dg * P:(dg + 1) * P, :])
                nc.vector.tensor_tensor(ht[:], ht[:], pb[:], mybir.AluOpType.mult)
                st = pp.tile([P, C], F32, tag=f"st{dg}")
                nc.vector.tensor_tensor_scan(
                    st[:], db[:], ht[:], 0.0,
                    mybir.AluOpType.mult, mybir.AluOpType.add)
                sts.append(st)

            # ---------------- chunk ids in wrapped-16 layout ----------------
            # CK16[q, j] = chunk_id[16j + q]. Host passes maskw[q, j] =
            # mask[16j + q]. Within-column inclusive prefix over q via the
            # 16x16 inclusive triangular matmul; across-column exclusive
            # prefix of the column sums via a 1-partition scan, broadcast
            # into the same PSUM bank with a ones matmul.
            maskw_sb = pp.tile([QW, FW], U8)
            nc.scalar.dma_start(maskw_sb[:], maskw_d[:])
            maskwf = pp.tile([QW, FW], F32)
            nc.vector.tensor_copy(maskwf[:], maskw_sb[:])
            ps16 = psone.tile([QW, FW], F32, space="PSUM", tag="ps16")
            nc.tensor.matmul(ps16[:], lhsT=uti[:], rhs=maskwf[:],
                             start=True, stop=False)
            # column sums at partition 0 via a ones matmul
            ones161 = pp.tile([QW, 1], F32)
            nc.vector.memset(ones161[:], 1.0)
            cs_ps = psone.tile([1, FW], F32, space="PSUM", tag="cs")
            nc.tensor.matmul(cs_ps[:], lhsT=ones161[:], rhs=maskwf[:],
                             start=True, stop=True)
            colsb = pp.tile([1, FW], F32)
            nc.vector.tensor_copy(colsb[:], cs_ps[:])
            ones1 = pp.tile([1, FW], F32)
            nc.vector.memset(ones1[:], 1.0)
            exc0 = pp.tile([1, FW], F32)
            nc.vector.tensor_tensor_scan(
                exc0[:], ones1[:], colsb[:],
                0.0, mybir.AluOpType.mult, mybir.AluOpType.add)
            # exclusive = inclusive - colsum
            nc.vector.tensor_tensor(exc0[:], exc0[:], colsb[:],
                                    mybir.AluOpType.subtract)
            nc.tensor.matmul(ps16[:], lhsT=ones16[:], rhs=exc0[:],
                             start=False, stop=True)
            ck = pp.tile([QW, FW], F32)
            nc.vector.tensor_scalar(ck[:], ps16[:], -1.0, None,
                                    mybir.AluOpType.add)
            nc.vector.tensor_scalar(ck[:], ck[:], 0.0, float(C - 1),
                                    mybir.AluOpType.max, mybir.AluOpType.min)
            ck16 = pp.tile([QW, FW], I16)
            nc.vector.tensor_copy(ck16[:], ck[:])
            # replicate to all 8 GPSIMD core groups (cross-partition copies)
            ckrep = pp.tile([P, FW], I16)
            for cgrp in range(P // QW):
                nc.scalar.dma_start(ckrep[cgrp * QW:(cgrp + 1) * QW, :], ck16[:])

            # transpose [D, C] -> [C, D] with 64 PE transposes (ci-major so
            # the first half of smoothed is ready early)
            for ci in range(NCB):
                for dg in range(NDG):
                    ps = psp.tile([P, P], F32, space="PSUM", tag="tps")
                    nc.tensor.transpose(ps[:], sts[dg][:, ci * P:(ci + 1) * P],
                                        ident[:])
                    nc.vector.tensor_copy(
                        sm_sb[:, ci * D + dg * P: ci * D + (dg + 1) * P], ps[:])

            # split smoothed writeback so the first gather can start after
            # the first half (tokens of call k only reference c < 1024(k+1))
            sm_dram = dp.tile([C, D], F32)
            sm_v = sm_dram[:].rearrange("(ci p) d -> p ci d", p=P)
            sb_v = sm_sb[:].rearrange("p (ci d) -> p ci d", d=D)
            HB = NCB // 2
            w_sm_a = nc.sync.dma_start(sm_v[:, 0:HB, :], sb_v[:, 0:HB, :])
            w_sm_b = nc.sync.dma_start(sm_v[:, HB:NCB, :], sb_v[:, HB:NCB, :])

            # ---------------- output expansion ----------------
            for k in range(NGC):
                g = gp.tile([P, IPC // P, D], F32, tag="g")
                g_i = nc.gpsimd.dma_gather(
                    out_ap=g[:], in_ap=sm_dram[:],
                    idxs_ap=ckrep[:, k * (FW // NGC):(k + 1) * (FW // NGC)],
                    num_idxs=IPC, num_idxs_reg=IPC, elem_size=D)
                bass._add_dep_helper(g_i.ins, w_sm_a.ins, sync=True,
                                     reason="smoothed gather raw a")
                if k > 0:
                    bass._add_dep_helper(g_i.ins, w_sm_b.ins, sync=True,
                                         reason="smoothed gather raw b")
                nc.sync.dma_start(
                    out_d[k * IPC:(k + 1) * IPC, :].rearrange(
                        "(g p) d -> p g d", p=P),
                    g[:])

    nc.compile()
    return nc


def _shard_inputs(hidden_states, boundary_mask, boundary_prob, take_idx):
    hidden_states = np.asarray(hidden_states, dtype=np.float32)
    boundary_mask = np.asarray(boundary_mask)
    boundary_prob = np.asarray(boundary_prob, dtype=np.float32)
    take_idx = np.asarray(take_idx)
    in_maps = []
    for b in range(B):
        in_maps.append({
            "hidden_t": np.ascontiguousarray(hidden_states[b].T),
            # maskw[q, j] = mask[16j + q]
            "maskw": np.ascontiguousarray(
                boundary_mask[b].astype(np.uint8).reshape(FW, QW).T),
            "prob": np.ascontiguousarray(boundary_prob[b].reshape(L, 1)),
            # (p, j) = take_idx[j*128 + p]
            "tidx": np.ascontiguousarray(
                take_idx[b].astype(np.int32).reshape(NPJ, P).T),
        })
    return in_maps


last_results = None  # populated by kernel() for profiling harnesses


def kernel(hidden_states, boundary_mask, boundary_prob, take_idx,
           **run_kwargs) -> np.ndarray:
    global _CACHED_NC, last_results
    if _CACHED_NC is None:
        _CACHED_NC = build_nc()
    in_maps = _shard_inputs(hidden_states, boundary_mask, boundary_prob, take_idx)
    res = run_bass_kernel_spmd(_CACHED_NC, in_maps, core_ids=list(range(B)),
                               **run_kwargs)
    last_results = res
    out = np.stack([np.asarray(res.results[b]["out"]) for b in range(B)], axis=0)
    return out.astype(np.float32, copy=False)



# revision 19
# speedup vs baseline: 1.2810x; 1.2810x over previous
"""Trainium2 Bass kernel for nn_DeChunkLayer (ragged_sequence).

Math (per batch row):
    p[c]     = clip(boundary_prob[take_idx[c]], EPS, 1-EPS)
    decay    = 1 - p, decay[0] = 0
    weighted = p * hidden, weighted[0] = hidden[0]
    smoothed[c] = decay[c] * smoothed[c-1] + weighted[c]      (EMA scan over C)
    chunk_id = clip(cumsum(boundary_mask) - 1, 0, C-1)
    out[l]   = smoothed[chunk_id[l]]

Device mapping (1 batch row per NeuronCore, 8 cores), bf16 data path
(fp32 scan state; rel tolerance is 2e-2, measured end-to-end ~4e-3):

  - p gather without GpSimd descriptors: take_idx is split hi=idx>>6 /
    lo=idx&63 on DVE; a [128hi x 128c] one-hot (iota compare of the
    broadcast hi row) matmuls against prob_w [128, 64] to pick rows, and
    an iota-compare one-hot over lo picks the column via mult+reduce.
  - EMA scan: DVE tensor_tensor_scan in transposed [D-part, C-free]
    layout on bf16 operands (hidden arrives bf16 pre-transposed from the
    host shard); internal scan state is fp32.
  - smoothed is PE-transposed back to [C, D] (bf16 PSUM copies) and
    written to a DRAM table of C+1 rows; row C duplicates row C-1 so the
    pair gather below stays in bounds at the chunk_id clip.
  - output expansion halves the SWDGE descriptor count with PAIR
    gathers: consecutive tokens 2k, 2k+1 have chunk ids cid[2k] and
    cid[2k]+mask[2k+1], so one descriptor (elem_size=1024 bf16,
    elem_step=512 -> overlapping 2-row reads at row cid[2k]) covers
    both. Even tokens DMA out directly from the gathered pair row;
    odd tokens are a copy + copy_predicated (mask = odd-token mask
    broadcast) then DMA. Output DRAM layout is [t, p, parity, d]
    (pair-major); the host unshuffles and upcasts to f32.
  - pair chunk ids come from the 16x16 triangular-matmul cumsum of the
    per-pair mask sums (pair-wrapped layout from the host), minus the
    odd mask, minus 1.
Tile does not emit RAW waits through DRAM scratch tensors, so every
DRAM-bounce reader carries an explicit sync dep on its writer.
"""

import numpy as np
import ml_dtypes

import concourse.bass as bass
import concourse.bacc as bacc
import concourse.mybir as mybir
import concourse.tile as tile
from concourse.bass_utils import run_bass_kernel_spmd
from concourse.masks import make_identity, make_upper_triangular

B, L, C, D = 8, 8192, 2048, 512
EPS = 1e-4
P = 128
NDG = D // P          # 4 partition groups of the transposed layout
NCB = C // P          # 16 c-blocks of 128
NPAIR = L // 2        # 4096 token pairs
QW = 16               # wrap width of the dma_gather index layout
PFW = NPAIR // QW     # 256 free positions in the pair-wrapped layout
NGC = 8               # gather calls
PPC = NPAIR // NGC    # 512 pairs per call
NT = NPAIR // P       # 32 pair-major 128-groups

F32 = mybir.dt.float32
BF16 = mybir.dt.bfloat16
I16 = mybir.dt.int16
I32 = mybir.dt.int32
U8 = mybir.dt.uint8

BFNP = ml_dtypes.bfloat16

_CACHED_NC = None


def build_nc() -> bacc.Bacc:
    nc = bacc.Bacc("TRN2", target_bir_lowering=False, debug=False)

    hidden_t = nc.dram_tensor("hidden_t", [D, C], BF16, kind="ExternalInput")
    prob_w_d = nc.dram_tensor("prob_w", [P, 64], BF16, kind="ExternalInput")
    tidx_d = nc.dram_tensor("tidx", [P, NCB], I32, kind="ExternalInput")
    me_d = nc.dram_tensor("me_w", [QW, PFW], U8, kind="ExternalInput")
    mo_d = nc.dram_tensor("mo_w", [QW, PFW], U8, kind="ExternalInput")
    mo128_d = nc.dram_tensor("mo128", [P, NT], U8, kind="ExternalInput")
    out_d = nc.dram_tensor("out16", [L, D], BF16, kind="ExternalOutput")

    with tile.TileContext(nc) as tc:
        with (
            tc.tile_pool(name="persist", bufs=1) as pp,
            tc.tile_pool(name="hid", bufs=2) as hp,
            tc.tile_pool(name="gather", bufs=3) as gp,
            tc.tile_pool(name="osel", bufs=3) as op_,
            tc.tile_pool(name="psum", bufs=2, space="PSUM") as psp,
            tc.tile_pool(name="psone", bufs=1, space="PSUM") as psone,
            tc.tile_pool(name="dram", bufs=1, space="DRAM") as dp,
        ):
            # ---------------- input loads ----------------
            tidx_sb = pp.tile([P, NCB], I32)
            nc.scalar.dma_start(tidx_sb[:], tidx_d[:])
            prob_sb = pp.tile([P, 64], BF16)
            nc.scalar.dma_start(prob_sb[:], prob_w_d[:])
            me_sb = pp.tile([QW, PFW], U8)
            nc.scalar.dma_start(me_sb[:], me_d[:])
            mo_sb = pp.tile([QW, PFW], U8)
            nc.scalar.dma_start(mo_sb[:], mo_d[:])
            mo128_sb = pp.tile([P, NT], U8)
            nc.scalar.dma_start(mo128_sb[:], mo128_d[:])

            # ---------------- constants ----------------
            # Masks built on Pool, then bounced through a DVE copy so matmuls
            # consuming them wait on a single (DVE) semaphore.
            ident_p = pp.tile([P, P], F32)
            make_identity(nc, ident_p[:])
            uti_p = pp.tile([QW, QW], F32)
            make_upper_triangular(nc, uti_p[:], val=1.0, diag=True)  # k <= m
            iotap_i = pp.tile([P, 1], I32)
            nc.gpsimd.iota(iotap_i[:], pattern=[[0, 1]], base=0,
                           channel_multiplier=1)
            iota64_i = pp.tile([P, 64], I32)
            nc.gpsimd.iota(iota64_i[:], pattern=[[1, 64]], base=0,
                           channel_multiplier=0)
            ident = pp.tile([P, P], F32)
            nc.vector.tensor_copy(ident[:], ident_p[:])
            ident16 = pp.tile([P, P], BF16)
            nc.vector.tensor_copy(ident16[:], ident_p[:])
            uti = pp.tile([QW, QW], F32)
            nc.vector.tensor_copy(uti[:], uti_p[:])
            iotap = pp.tile([P, 1], F32)
            nc.vector.tensor_copy(iotap[:], iotap_i[:])
            iota64 = pp.tile([P, 64], F32)
            nc.vector.tensor_copy(iota64[:], iota64_i[:])
            ones1x128 = pp.tile([1, P], BF16)
            nc.vector.memset(ones1x128[:], 1.0)
            ones16 = pp.tile([1, QW], F32)
            nc.vector.memset(ones16[:], 1.0)
            ones161 = pp.tile([QW, 1], F32)
            nc.vector.memset(ones161[:], 1.0)
            ones1f = pp.tile([1, PFW], F32)
            nc.vector.memset(ones1f[:], 1.0)

            # ---------------- p extraction (one-hot matmuls) ----------------
            # hi = tidx >> 6, lo = tidx & 63 (exact int ops)
            hi_i = pp.tile([P, NCB], I32)
            nc.vector.tensor_scalar(hi_i[:], tidx_sb[:], 6, None,
                                    mybir.AluOpType.arith_shift_right)
            lo_i = pp.tile([P, NCB], I32)
            nc.vector.tensor_scalar(lo_i[:], tidx_sb[:], 63, None,
                                    mybir.AluOpType.bitwise_and)
            hif = pp.tile([P, NCB], BF16)   # hi <= 127: exact in bf16
            nc.vector.tensor_copy(hif[:], hi_i[:])
            lof = pp.tile([P, NCB], F32)
            nc.vector.tensor_copy(lof[:], lo_i[:])
            hiT_ps = psp.tile([P, P], BF16, space="PSUM", tag="tps")
            nc.tensor.transpose(hiT_ps[0:NCB, :], hif[:], ident16[:])
            hiT = pp.tile([NCB, P], BF16)
            nc.vector.tensor_copy(hiT[:], hiT_ps[0:NCB, :])
            # matmul rhs needs base partition 0: flatten rows onto one row
            hiTr = pp.tile([1, C], BF16)
            nc.scalar.dma_start(hiTr[:], hiT[:])

            p2 = pp.tile([P, NCB], F32)   # (p, j) = p value for c = j*128 + p
            for ci in range(NCB):
                bc_ps = psp.tile([P, P], F32, space="PSUM", tag="bc", bufs=1)
                nc.tensor.matmul(bc_ps[:], lhsT=ones1x128[:],
                                 rhs=hiTr[:, ci * P:(ci + 1) * P],
                                 start=True, stop=True)
                oh1 = pp.tile([P, P], BF16, tag=f"oh1_{ci % 2}")
                nc.vector.tensor_scalar(oh1[:], bc_ps[:], iotap[:], None,
                                        mybir.AluOpType.is_equal)
                r_ps = psp.tile([P, 64], F32, space="PSUM", tag="rps", bufs=1)
                nc.tensor.matmul(r_ps[:], lhsT=oh1[:], rhs=prob_sb[:],
                                 start=True, stop=True)
                oh64 = pp.tile([P, 64], F32, tag=f"oh64_{ci % 2}")
                nc.vector.tensor_scalar(oh64[:], iota64[:], lof[:, ci:ci + 1],
                                        None, mybir.AluOpType.is_equal)
                nc.vector.tensor_tensor(oh64[:], oh64[:], r_ps[:],
                                        mybir.AluOpType.mult)
                nc.vector.tensor_reduce(p2[:, ci:ci + 1], oh64[:],
                                        mybir.AxisListType.X,
                                        mybir.AluOpType.add)

            nc.vector.tensor_scalar(p2[:], p2[:], EPS, 1.0 - EPS,
                                    mybir.AluOpType.max, mybir.AluOpType.min)
            d2 = pp.tile([P, NCB], F32)
            nc.vector.tensor_scalar(d2[:], p2[:], -1.0, 1.0,
                                    mybir.AluOpType.mult, mybir.AluOpType.add)
            nc.vector.memset(d2[0:1, 0:1], 0.0)   # decay[0] = 0
            nc.vector.memset(p2[0:1, 0:1], 1.0)   # weighted[0] = hidden[0]

            # p/decay rows: transpose to [16, 128] (contiguous c per
            # partition), write to DRAM, broadcast-read into [128, C] bf16.
            pT_ps = psone.tile([NCB, P], F32, space="PSUM", tag="prow")
            nc.tensor.transpose(pT_ps[:], p2[:], ident[:])
            dT_ps = psone.tile([NCB, P], F32, space="PSUM", tag="drow")
            nc.tensor.transpose(dT_ps[:], d2[:], ident[:])
            pT = pp.tile([NCB, P], BF16)
            nc.vector.tensor_copy(pT[:], pT_ps[:])
            dT = pp.tile([NCB, P], BF16)
            nc.vector.tensor_copy(dT[:], dT_ps[:])

            p_dram = dp.tile([1, C], BF16)
            d_dram = dp.tile([1, C], BF16)
            w_p = nc.scalar.dma_start(
                p_dram[:].rearrange("o (j q) -> (o j) q", j=NCB), pT[:])
            w_d = nc.scalar.dma_start(
                d_dram[:].rearrange("o (j q) -> (o j) q", j=NCB), dT[:])
            pb = pp.tile([P, C], BF16)
            db = pp.tile([P, C], BF16)
            r_p = nc.scalar.dma_start(pb[:], p_dram[:].to_broadcast([P, C]))
            r_d = nc.scalar.dma_start(db[:], d_dram[:].to_broadcast([P, C]))
            bass._add_dep_helper(r_p.ins, w_p.ins, sync=True, reason="p bounce raw")
            bass._add_dep_helper(r_d.ins, w_d.ins, sync=True, reason="d bounce raw")

            # ---------------- pair chunk ids ----------------
            # cide[s] = cumsum_pairs_incl(me+mo)[s] - mo[s] - 1, clipped to
            # C-1, in the pair-wrapped-16 layout W[q, i] = x[16i + q].
            mef = pp.tile([QW, PFW], F32)
            nc.vector.tensor_copy(mef[:], me_sb[:])
            mof = pp.tile([QW, PFW], F32)
            nc.vector.tensor_copy(mof[:], mo_sb[:])
            pairsum = pp.tile([QW, PFW], F32)
            nc.vector.tensor_tensor(pairsum[:], mef[:], mof[:],
                                    mybir.AluOpType.add)
            ps16 = psone.tile([QW, PFW], F32, space="PSUM", tag="ps16")
            nc.tensor.matmul(ps16[:], lhsT=uti[:], rhs=pairsum[:],
                             start=True, stop=False)
            cs_ps = psone.tile([1, PFW], F32, space="PSUM", tag="cs")
            nc.tensor.matmul(cs_ps[:], lhsT=ones161[:], rhs=pairsum[:],
                             start=True, stop=True)
            colsb = pp.tile([1, PFW], F32)
            nc.vector.tensor_copy(colsb[:], cs_ps[:])
            exc0 = pp.tile([1, PFW], F32)
            nc.vector.tensor_tensor_scan(
                exc0[:], ones1f[:], colsb[:],
                0.0, mybir.AluOpType.mult, mybir.AluOpType.add)
            nc.vector.tensor_tensor(exc0[:], exc0[:], colsb[:],
                                    mybir.AluOpType.subtract)
            nc.tensor.matmul(ps16[:], lhsT=ones16[:], rhs=exc0[:],
                             start=False, stop=True)
            ck = pp.tile([QW, PFW], F32)
            nc.vector.tensor_tensor(ck[:], ps16[:], mof[:],
                                    mybir.AluOpType.subtract)
            nc.vector.tensor_scalar(ck[:], ck[:], -1.0, float(C - 1),
                                    mybir.AluOpType.add, mybir.AluOpType.min)
            ck16 = pp.tile([QW, PFW], I16)
            nc.vector.tensor_copy(ck16[:], ck[:])
            # replicate to all 8 GPSIMD core groups (cross-partition copies)
            ckrep = pp.tile([P, PFW], I16)
            for cgrp in range(P // QW):
                nc.scalar.dma_start(ckrep[cgrp * QW:(cgrp + 1) * QW, :], ck16[:])


            # ---------------- EMA scan in transposed layout ----------------
            sts = []
            for dg in range(NDG):
                ht = hp.tile([P, C], BF16, tag=f"ht{dg % 2}")
                nc.sync.dma_start(ht[:], hidden_t[dg * P:(dg + 1) * P, :])
                nc.vector.tensor_tensor(ht[:], ht[:], pb[:], mybir.AluOpType.mult)
                st = pp.tile([P, C], BF16, tag=f"st{dg}")
                nc.vector.tensor_tensor_scan(
                    st[:], db[:], ht[:], 0.0,
                    mybir.AluOpType.mult, mybir.AluOpType.add)
                sts.append(st)

            # transpose [D, C] -> [C, D] with 64 PE transposes (ci-major so
            # the early table rows are ready first), PSUM copies cast to bf16
            sm_sb = pp.tile([P, NCB * D], BF16)  # [c-in-block, (c-block, d)]
            for ci in range(NCB):
                for dg in range(NDG):
                    ps = psp.tile([P, P], BF16, space="PSUM", tag="tps")
                    nc.tensor.transpose(ps[:], sts[dg][:, ci * P:(ci + 1) * P],
                                        ident16[:])
                    nc.vector.tensor_copy(
                        sm_sb[:, ci * D + dg * P: ci * D + (dg + 1) * P], ps[:])

            # table writes: rows [0, 1152) unblock gather call 0; row C
            # duplicates row C-1 (pair reads at a clipped chunk id stay
            # correct: both slots then hold smoothed[C-1]).
            sm16_d = dp.tile([C + 1, D], BF16)
            sm_v = sm16_d[0:C, :].rearrange("(ci p) d -> p ci d", p=P)
            sb_v = sm_sb[:].rearrange("p (ci d) -> p ci d", d=D)
            HB = 9
            w_sm_a = nc.sync.dma_start(sm_v[:, 0:HB, :], sb_v[:, 0:HB, :])
            w_sm_b = nc.sync.dma_start(sm_v[:, HB:NCB, :], sb_v[:, HB:NCB, :])
            w_dup = nc.sync.dma_start(
                sm16_d[C:C + 1, :], sm_sb[P - 1:P, (NCB - 1) * D:NCB * D])

            # ---------------- output expansion (pair gathers) ----------------
            tbl = sm16_d[:]
            gather_in = bass.AP(tbl.tensor, tbl.offset, [[D, C], [1, 2 * D]])
            out_v = out_d[:].rearrange("(t p two) d -> p t two d", p=P, two=2)
            TPC = PPC // P  # 4 pair-major 128-groups per call
            for k in range(NGC):
                g = gp.tile([P, TPC, 2 * D], BF16, tag="g")
                g_i = nc.gpsimd.dma_gather(
                    out_ap=g[:], in_ap=gather_in,
                    idxs_ap=ckrep[:, k * (PFW // NGC):(k + 1) * (PFW // NGC)],
                    num_idxs=PPC, num_idxs_reg=PPC, elem_size=2 * D,
                    elem_step=D)
                bass._add_dep_helper(g_i.ins, w_sm_a.ins, sync=True,
                                     reason="table raw a")
                if k >= 1:
                    bass._add_dep_helper(g_i.ins, w_sm_b.ins, sync=True,
                                         reason="table raw b")
                if k >= 2:
                    bass._add_dep_helper(g_i.ins, w_dup.ins, sync=True,
                                         reason="table raw dup")
                # even tokens: straight DMA of the first row of each pair
                nc.sync.dma_start(out_v[:, k * TPC:(k + 1) * TPC, 0, :],
                                  g[:, :, 0:D])
                # odd tokens: select second row where mask[2k+1]
                o = op_.tile([P, TPC, D], BF16, tag="o")
                nc.vector.tensor_copy(o[:], g[:, :, 0:D])
                for gi in range(TPC):
                    mask_ap = (mo128_sb[:, k * TPC + gi:k * TPC + gi + 1]
                               .to_broadcast([P, D]))
                    nc.vector.copy_predicated(o[:, gi, :], mask_ap,
                                              g[:, gi, D:2 * D])
                nc.sync.dma_start(out_v[:, k * TPC:(k + 1) * TPC, 1, :], o[:])

    nc.compile()
    return nc


def _shard_inputs(hidden_states, boundary_mask, boundary_prob, take_idx):
    hidden_states = np.asarray(hidden_states, dtype=np.float32)
    boundary_mask = np.asarray(boundary_mask)
    boundary_prob = np.asarray(boundary_prob, dtype=np.float32)
    take_idx = np.asarray(take_idx)
    in_maps = []
    for b in range(B):
        mask = boundary_mask[b].astype(np.uint8)
        # pair-wrapped even/odd masks: W[q, i] = x[16i + q]
        me = mask[0::2].reshape(PFW, QW).T
        mo = mask[1::2].reshape(PFW, QW).T
        in_maps.append({
            "hidden_t": np.ascontiguousarray(hidden_states[b].T).astype(BFNP),
            "prob_w": boundary_prob[b].reshape(P, 64).astype(BFNP),
            # (p, j) = take_idx[j*128 + p]
            "tidx": np.ascontiguousarray(
                take_idx[b].astype(np.int32).reshape(NCB, P).T),
            "me_w": np.ascontiguousarray(me),
            "mo_w": np.ascontiguousarray(mo),
            # (p, t) = mask[2*(128t + p) + 1]
            "mo128": np.ascontiguousarray(mask[1::2].reshape(NT, P).T),
        })
    return in_maps


last_results = None  # populated by kernel() for profiling harnesses


def kernel(hidden_states, boundary_mask, boundary_prob, take_idx,
           **run_kwargs) -> np.ndarray:
    global _CACHED_NC, last_results
    if _CACHED_NC is None:
        _CACHED_NC = build_nc()
    in_maps = _shard_inputs(hidden_states, boundary_mask, boundary_prob, take_idx)
    res = run_bass_kernel_spmd(_CACHED_NC, in_maps, core_ids=list(range(B)),
                               **run_kwargs)
    last_results = res
    # the (t p two) device view writes DRAM row 256t + 2p + parity, which
    # equals the token index: out16 is already in token order
    out = np.stack(
        [np.asarray(res.results[b]["out16"]) for b in range(B)], axis=0)
    return out.astype(np.float32)
